# revision 24
# baseline (speedup 1.0000x reference)
"""Paged GQA chunked-prefill attention for 8 Trainium2 NeuronCores.

Problem (hardcoded): B=4 seqs x Q=256 new tokens, H=32 query heads, KVH=8 kv
heads (GQA group G=4), D=128 head dim, paged KV cache of 512 blocks x 16
tokens, per-seq lengths in seq_lens (clamped to >= Q), causal masking.

Sharding: tensor-parallel over heads. Core h gets kv head h and query heads
h*4..h*4+3; block_tables/seq_lens are resolved host-side while packing the
shards; the output is all-gathered host-side over the hidden dim.

Per-core device kernel (seq b, kv chunk c of 128 positions, q = (g,t) -> 1024
columns, processed in two 512-column halves n):
  S^T[kv,qh] = K_c^T q            (f32r matmul, full PE rate)
  S^T += causal mask              (identity-lhsT matmul into the same bank)
  U = exp(SCALE * S^T)            (ScalarE, PSUM->SBUF, float32r out)
  l[2,qh] += ones2^T @ U          (wide denominator matmul, q stays on free)
  O^T[d,qh] += V_c^T @ U          (PSUM accumulation over chunks)
Per-seq epilogue: l -> SBUF (ScalarE), PE-transpose l to [128,8] and O^T to
[q,d], rlt = 1/l (VectorE), out = O * rlt (tensor_scalar), DMA out.

Sequences are processed longest-first so the PE warms up on the big unmasked
run while the remaining DMAs and the mask constants stream in.
"""
import math

import ml_dtypes
import numpy as np

import concourse.mybir as mybir
import concourse.tile as tile
from concourse import bacc
from concourse.bass_utils import run_bass_kernel_spmd

B, Q, H, D = 4, 256, 32, 128
KVH = 8
G = H // KVH
BLOCK = 16
NB = 128
KV = NB * BLOCK
NUM_BLOCKS = B * NB
SCALE = 1.0 / math.sqrt(D)
N_CORES = 8
CHUNK = 128
QCOLS = G * Q  # 1024 q columns per sequence per core
NHALF = 512

F32 = mybir.dt.float32
F32R = mybir.dt.float32r
NEG = -1.0e9


def _plan(seq_lens):
    """Per-seq chunk counts, offsets, and boundary-chunk mask tiles."""
    L = np.maximum(np.asarray(seq_lens, dtype=np.int64), Q)
    cb = [int((int(Lb) + CHUNK - 1) // CHUNK) for Lb in L]
    offs = np.concatenate([[0], np.cumsum(cb)]).astype(int)
    masked = []  # list of (b, c, mask[128,256])
    t = np.arange(Q)
    p = np.arange(CHUNK)
    for b in range(B):
        Lb = int(L[b])
        for c in range(cb[b]):
            if c * CHUNK + CHUNK - 1 > Lb - Q:
                kvpos = c * CHUNK + p
                m = np.where(
                    kvpos[:, None] > (Lb - Q) + t[None, :], NEG, 0.0
                ).astype(np.float32)
                masked.append((b, c, m))
    return L, cb, offs, masked


def _build(seq_lens):
    L, cb, offs, masked = _plan(seq_lens)
    C = int(offs[-1])
    nmask = len(masked)
    border = sorted(range(B), key=lambda b: cb[b])  # shortest first
    # order mask tiles by processing order so the early ones land first
    order = sorted(range(len(masked)), key=lambda i: (border.index(masked[i][0]), masked[i][1]))
    masked = [masked[i] for i in order]
    mask_np = np.concatenate([m for _, _, m in masked], axis=1).astype(
        ml_dtypes.bfloat16
    )  # [128, nm*256]; 0/-1e9 are bf16-exact
    mask_idx = {(b, c): i for i, (b, c, _) in enumerate(masked)}
    ident_np = np.eye(CHUNK, dtype=np.float32)
    identb_np = np.eye(CHUNK, dtype=ml_dtypes.bfloat16)
    ones_np = np.ones((CHUNK, 2), dtype=np.float32)

    nc = bacc.Bacc(
        "TRN2", target_bir_lowering=False, debug=False, num_devices=N_CORES
    )
    kt_d = nc.dram_tensor("kt", [D, C * CHUNK], F32R, kind="ExternalInput")
    v_d = nc.dram_tensor("v", [CHUNK, C * CHUNK], F32R, kind="ExternalInput")
    qt_d = nc.dram_tensor("qt", [D, B * QCOLS], F32R, kind="ExternalInput")
    out_d = nc.dram_tensor("out", [B, D, QCOLS], F32, kind="ExternalOutput")
    mask_d = nc.inline_tensor(mask_np, name="mask_const")
    identb_d = nc.inline_tensor(identb_np, name="identb_const")
    ident_d = nc.inline_tensor(ident_np, name="ident_const")
    ones_d = nc.inline_tensor(ones_np, name="ones_const")

    exp = mybir.ActivationFunctionType.Exp

    with tile.TileContext(nc) as tc:
        with (
            tc.tile_pool(name="sbin", bufs=1) as sbin,
            tc.tile_pool(name="sbu", bufs=6) as sbu,
            tc.tile_pool(name="sbe", bufs=3) as sbe,
            tc.tile_pool(name="ps_s", bufs=4, space="PSUM") as ps_s,
            tc.tile_pool(name="ps_o", bufs=1, space="PSUM") as ps_o,
            tc.tile_pool(name="ps_l", bufs=1, space="PSUM") as ps_l,
        ):
            # Critical-path DMAs first: K chunk 0 / first q half of the
            # first (longest) sequence, so the PE starts ~10us earlier.
            b0 = border[0]
            kt_t = [None] * B
            qt_t = [None] * B
            v_t = [None] * B
            w0 = cb[b0] * CHUNK
            kt_first = sbin.tile([D, w0], F32R, tag=f"kt{b0}")
            nc.sync.dma_start(
                kt_first[:, 0:CHUNK],
                kt_d.ap()[:, offs[b0] * CHUNK : offs[b0] * CHUNK + CHUNK],
            )
            qt_first = sbin.tile([D, QCOLS], F32R, tag=f"qt{b0}")
            nc.sync.dma_start(
                qt_first[:, 0:NHALF],
                qt_d.ap()[:, b0 * QCOLS : b0 * QCOLS + NHALF],
            )
            nc.sync.dma_start(
                qt_first[:, NHALF:QCOLS],
                qt_d.ap()[:, b0 * QCOLS + NHALF : (b0 + 1) * QCOLS],
            )
            v_first = sbin.tile([CHUNK, w0], F32R, tag=f"v{b0}")
            nc.sync.dma_start(
                v_first[:, 0 : 2 * CHUNK],
                v_d.ap()[:, offs[b0] * CHUNK : offs[b0] * CHUNK + 2 * CHUNK],
            )
            kcut = CHUNK
            vcut = 2 * CHUNK
            while kcut < w0 or vcut < w0:
                khi = min(kcut + 4 * CHUNK, w0)
                if khi > kcut:
                    nc.sync.dma_start(
                        kt_first[:, kcut:khi],
                        kt_d.ap()[
                            :, offs[b0] * CHUNK + kcut : offs[b0] * CHUNK + khi
                        ],
                    )
                    kcut = khi
                vhi = min(vcut + 4 * CHUNK, w0)
                if vhi > vcut:
                    nc.sync.dma_start(
                        v_first[:, vcut:vhi],
                        v_d.ap()[
                            :, offs[b0] * CHUNK + vcut : offs[b0] * CHUNK + vhi
                        ],
                    )
                    vcut = vhi
            kt_t[b0] = kt_first
            qt_t[b0] = qt_first

            identr = sbin.tile([CHUNK, CHUNK], mybir.dt.bfloat16, tag="identr")
            nc.sync.dma_start(identr[:], identb_d.ap())
            ones = sbin.tile([CHUNK, 2], F32R, tag="ones")
            nc.gpsimd.dma_start(ones[:], ones_d.ap())
            masks = sbin.tile([CHUNK, nmask * Q], mybir.dt.bfloat16, tag="masks")
            cut = Q * sum(
                1 for bb, _, _ in masked if cb[bb] <= cb[border[1]]
            )
            cut = max(Q, min(cut, nmask * Q))
            nc.sync.dma_start(masks[:, 0:cut], mask_d.ap()[:, 0:cut])
            if cut < nmask * Q:
                nc.sync.dma_start(
                    masks[:, cut : nmask * Q], mask_d.ap()[:, cut : nmask * Q]
                )

            for b in border:
                w = cb[b] * CHUNK
                head = min(2 * CHUNK, w)
                o0 = offs[b] * CHUNK
                if b == border[0]:
                    v_t[b] = v_first
                    continue
                vt = sbin.tile([CHUNK, w], F32R, tag=f"v{b}")
                if kt_t[b] is None:
                    kt = sbin.tile([D, w], F32R, tag=f"kt{b}")
                    nc.sync.dma_start(
                        kt[:, 0:head], kt_d.ap()[:, o0 : o0 + head]
                    )
                    qt = sbin.tile([D, QCOLS], F32R, tag=f"qt{b}")
                    nc.sync.dma_start(
                        qt[:], qt_d.ap()[:, b * QCOLS : (b + 1) * QCOLS]
                    )
                    nc.sync.dma_start(
                        vt[:, 0:head], v_d.ap()[:, o0 : o0 + head]
                    )
                    if head < w:
                        nc.sync.dma_start(
                            kt[:, head:w], kt_d.ap()[:, o0 + head : o0 + w]
                        )
                        nc.sync.dma_start(
                            vt[:, head:w], v_d.ap()[:, o0 + head : o0 + w]
                        )
                    kt_t[b] = kt
                    qt_t[b] = qt
                else:
                    nc.sync.dma_start(
                        vt[:, 0:head], v_d.ap()[:, o0 : o0 + head]
                    )
                    if head < w:
                        nc.sync.dma_start(
                            vt[:, head:w], v_d.ap()[:, o0 + head : o0 + w]
                        )
                v_t[b] = vt

            def half_state(b, c, n):
                # 'skip' = every q in the half is masked for this chunk;
                # 'mask' = the causal diagonal crosses this (chunk, half)
                lo = int(L[b]) - Q + n * CHUNK
                if c * CHUNK > lo + CHUNK - 1:
                    return "skip"
                if c * CHUNK + CHUNK - 1 > lo:
                    return "mask"
                return "clear"

            def emit_score(b, c):
                mi = mask_idx.get((b, c))
                u_h = []
                for n in range(2):
                    st = half_state(b, c, n)
                    if st == "skip":
                        u_h.append(None)
                        continue
                    s_ps = ps_s.tile([CHUNK, NHALF], F32, tag="s")
                    nc.tensor.matmul(
                        s_ps[:],
                        kt_t[b][:, c * CHUNK : (c + 1) * CHUNK],
                        qt_t[b][:, n * NHALF : (n + 1) * NHALF],
                        start=True,
                        stop=st == "clear",
                    )
                    if st == "mask":
                        mb = (
                            masks[
                                :,
                                mi * Q + n * CHUNK : mi * Q + (n + 1) * CHUNK,
                            ]
                            .unsqueeze(2)
                            .broadcast_to([CHUNK, CHUNK, G])
                        )
                        nc.tensor.matmul(
                            s_ps[:], identr[:], mb, start=False, stop=True
                        )
                    u = sbu.tile([CHUNK, NHALF], F32R, tag="u")
                    nc.scalar.activation(u[:], s_ps[:], exp, scale=SCALE)
                    u_h.append(u)
                return u_h

            def emit_consume(b, c, u_h, o_ps, l_ps, last_n):
                for n in range(2):
                    if u_h[n] is None:
                        continue
                    nc.tensor.matmul(
                        l_ps[:, n * NHALF : (n + 1) * NHALF],
                        ones[:, 0:2],
                        u_h[n][:],
                        start=c == 0,
                        stop=c == last_n[n],
                    )
                for n in range(2):
                    if u_h[n] is None:
                        continue
                    nc.tensor.matmul(
                        o_ps[:, n * NHALF : (n + 1) * NHALF],
                        v_t[b][:, c * CHUNK : (c + 1) * CHUNK],
                        u_h[n][:],
                        start=c == 0,
                        stop=c == last_n[n],
                    )

            u0_next = None
            for bi, b in enumerate(border):
                terminal = bi == len(border) - 1
                nchunks = cb[b]
                # last contributing chunk per half (later ones are skipped)
                last_n = [
                    min(nchunks - 1, (int(L[b]) - Q + n * CHUNK + CHUNK - 1) // CHUNK)
                    for n in range(2)
                ]
                o_ps = ps_o.tile([D, QCOLS], F32, tag="o")
                l_ps = ps_l.tile([2, QCOLS], F32, tag="l")
                for c in range(nchunks):
                    if c == 0 and u0_next is not None:
                        u_h = u0_next
                        u0_next = None
                    else:
                        u_h = emit_score(b, c)
                    emit_consume(b, c, u_h, o_ps, l_ps, last_n)
                # prologue: next sequence's first score chunk keeps ScalarE
                # fed across the transition
                if not terminal:
                    u0_next = emit_score(border[border.index(b) + 1], 0)

                # epilogue: rl = 1/l broadcast down partitions, one multiply.
                # o is copied out of PSUM immediately so the next sequence's
                # PV accumulation can claim the banks.
                l_sb = sbe.tile([1, QCOLS], F32, tag="lsb")
                if terminal:
                    # tail chain: l-copy on the (now idle) ScalarE, and read
                    # O straight from PSUM -- no next sequence needs the banks
                    nc.scalar.copy(l_sb[:], l_ps[0:1, :])
                    osrc = o_ps
                else:
                    nc.vector.tensor_copy(l_sb[:], l_ps[0:1, :])
                    ocp = sbe.tile([D, QCOLS], F32, tag="ocp")
                    nc.vector.tensor_copy(ocp[:], o_ps[:])
                    osrc = ocp
                rl_row = sbe.tile([1, QCOLS], F32, tag="rlrow")
                rlb = sbe.tile([D, QCOLS], F32, tag="rlb")
                out_sb = sbe.tile([D, QCOLS], F32, tag="osb")
                for n in range(2):
                    half = slice(n * NHALF, (n + 1) * NHALF)
                    nc.vector.reciprocal_approx_fast(
                        rl_row[:, half], l_sb[:, half]
                    )
                    nc.gpsimd.partition_broadcast(
                        rlb[:, half], rl_row[:, half]
                    )
                    nc.vector.tensor_mul(
                        out_sb[:, half], osrc[:, half], rlb[:, half]
                    )
                    nc.sync.dma_start(
                        out_d.ap()[b][:, half], out_sb[:, half]
                    )

    nc.compile()
    return nc, L, cb, offs


def _pack_inputs(query, k_cache, v_cache, block_tables, L, cb, offs):
    """Gather the paged cache and pack per-core shards in device layouts."""
    C = int(offs[-1])
    k_lin = k_cache[block_tables].reshape(B, KV, KVH, D)
    v_lin = v_cache[block_tables].reshape(B, KV, KVH, D)
    kt_all = np.zeros((KVH, D, C * CHUNK), dtype=np.float32)
    v_all = np.zeros((KVH, CHUNK, C * CHUNK), dtype=np.float32)
    for b in range(B):
        Lb, w = int(L[b]), cb[b] * CHUNK
        kk = np.zeros((w, KVH, D), dtype=np.float32)
        kk[:Lb] = k_lin[b, :Lb]
        # [w, KVH, D] -> [KVH, D, w]
        kt_all[:, :, offs[b] * CHUNK : offs[b] * CHUNK + w] = kk.transpose(
            1, 2, 0
        )
        vv = np.zeros((w, KVH, D), dtype=np.float32)
        vv[:Lb] = v_lin[b, :Lb]
        # [cb, 128, KVH, D] -> [KVH, 128, cb, D] -> [KVH, 128, w]
        v_all[:, :, offs[b] * CHUNK : offs[b] * CHUNK + w] = (
            vv.reshape(cb[b], CHUNK, KVH, D)
            .transpose(2, 1, 0, 3)
            .reshape(KVH, CHUNK, w)
        )
    # query [B,Q,H,D] -> [KVH, D, B, Q, G] (t-major, g inner)
    qt_all = (
        query.transpose(2, 3, 0, 1)
        .reshape(KVH, G, D, B, Q)
        .transpose(0, 2, 3, 4, 1)
        .reshape(KVH, D, B * QCOLS)
    )
    qt_all = np.ascontiguousarray(qt_all, dtype=np.float32)
    return [
        {
            "kt": np.ascontiguousarray(kt_all[h]),
            "v": np.ascontiguousarray(v_all[h]),
            "qt": qt_all[h],
        }
        for h in range(KVH)
    ]


def _unpack_outputs(results):
    """[B,D,QCOLS] per core (O^T, q=(g,t) on cols) -> [B*Q, H*D]."""
    out = np.empty((B * Q, H * D), dtype=np.float32)
    for h, res in enumerate(results):
        o = res["out"].reshape(B, D, Q, G)  # [b, d, t, g]
        o = o.transpose(0, 2, 3, 1).reshape(B * Q, G * D)
        out[:, h * G * D : (h + 1) * G * D] = o
    return out


def kernel(query, k_cache, v_cache, block_tables, seq_lens):
    query = np.asarray(query, dtype=np.float32)
    k_cache = np.asarray(k_cache, dtype=np.float32)
    v_cache = np.asarray(v_cache, dtype=np.float32)
    block_tables = np.asarray(block_tables, dtype=np.int64)
    nc, L, cb, offs = _build(np.asarray(seq_lens))
    in_maps = _pack_inputs(query, k_cache, v_cache, block_tables, L, cb, offs)
    res = run_bass_kernel_spmd(nc, in_maps, core_ids=list(range(N_CORES)))
    return _unpack_outputs(res.results)


# revision 25
# speedup vs baseline: 1.0035x; 1.0035x over previous
"""Paged GQA chunked-prefill attention for 8 Trainium2 NeuronCores.

Problem (hardcoded): B=4 seqs x Q=256 new tokens, H=32 query heads, KVH=8 kv
heads (GQA group G=4), D=128 head dim, paged KV cache of 512 blocks x 16
tokens, per-seq lengths in seq_lens (clamped to >= Q), causal masking.

Sharding: tensor-parallel over heads. Core h gets kv head h and query heads
h*4..h*4+3; block_tables/seq_lens are resolved host-side while packing the
shards; the output is all-gathered host-side over the hidden dim.

Per-core device kernel (seq b, kv chunk c of 128 positions, q = (g,t) -> 1024
columns, processed in two 512-column halves n):
  S^T[kv,qh] = K_c^T q            (f32r matmul, full PE rate)
  S^T += causal mask              (identity-lhsT matmul into the same bank)
  U = exp(SCALE * S^T)            (ScalarE, PSUM->SBUF, float32r out)
  l[2,qh] += ones2^T @ U          (wide denominator matmul, q stays on free)
  O^T[d,qh] += V_c^T @ U          (PSUM accumulation over chunks)
Per-seq epilogue: l -> SBUF (ScalarE), PE-transpose l to [128,8] and O^T to
[q,d], rlt = 1/l (VectorE), out = O * rlt (tensor_scalar), DMA out.

Sequences are processed longest-first so the PE warms up on the big unmasked
run while the remaining DMAs and the mask constants stream in.
"""
import math

import ml_dtypes
import numpy as np

import concourse.mybir as mybir
import concourse.tile as tile
from concourse import bacc
from concourse.bass_utils import run_bass_kernel_spmd

B, Q, H, D = 4, 256, 32, 128
KVH = 8
G = H // KVH
BLOCK = 16
NB = 128
KV = NB * BLOCK
NUM_BLOCKS = B * NB
SCALE = 1.0 / math.sqrt(D)
N_CORES = 8
CHUNK = 128
QCOLS = G * Q  # 1024 q columns per sequence per core
NHALF = 512

F32 = mybir.dt.float32
F32R = mybir.dt.float32r
NEG = -1.0e9


def _plan(seq_lens):
    """Per-seq chunk counts, offsets, and boundary-chunk mask tiles."""
    L = np.maximum(np.asarray(seq_lens, dtype=np.int64), Q)
    cb = [int((int(Lb) + CHUNK - 1) // CHUNK) for Lb in L]
    offs = np.concatenate([[0], np.cumsum(cb)]).astype(int)
    masked = []  # list of (b, c, mask[128,256])
    t = np.arange(Q)
    p = np.arange(CHUNK)
    for b in range(B):
        Lb = int(L[b])
        for c in range(cb[b]):
            if c * CHUNK + CHUNK - 1 > Lb - Q:
                kvpos = c * CHUNK + p
                m = np.where(
                    kvpos[:, None] > (Lb - Q) + t[None, :], NEG, 0.0
                ).astype(np.float32)
                masked.append((b, c, m))
    return L, cb, offs, masked


def _build(seq_lens):
    L, cb, offs, masked = _plan(seq_lens)
    C = int(offs[-1])
    nmask = len(masked)
    border = sorted(range(B), key=lambda b: cb[b])  # shortest first
    # order mask tiles by processing order so the early ones land first
    order = sorted(range(len(masked)), key=lambda i: (border.index(masked[i][0]), masked[i][1]))
    masked = [masked[i] for i in order]
    mask_np = np.concatenate([m for _, _, m in masked], axis=1).astype(
        ml_dtypes.bfloat16
    )  # [128, nm*256]; 0/-1e9 are bf16-exact
    mask_idx = {(b, c): i for i, (b, c, _) in enumerate(masked)}
    ident_np = np.eye(CHUNK, dtype=np.float32)
    identb_np = np.eye(CHUNK, dtype=ml_dtypes.bfloat16)
    ones_np = np.ones((CHUNK, 2), dtype=np.float32)

    nc = bacc.Bacc(
        "TRN2", target_bir_lowering=False, debug=False, num_devices=N_CORES
    )
    kt_d = nc.dram_tensor("kt", [D, C * CHUNK], F32R, kind="ExternalInput")
    v_d = nc.dram_tensor("v", [CHUNK, C * CHUNK], F32R, kind="ExternalInput")
    qt_d = nc.dram_tensor("qt", [D, B * QCOLS], F32R, kind="ExternalInput")
    out_d = nc.dram_tensor("out", [B, D, QCOLS], F32, kind="ExternalOutput")
    mask_d = nc.inline_tensor(mask_np, name="mask_const")
    identb_d = nc.inline_tensor(identb_np, name="identb_const")
    ident_d = nc.inline_tensor(ident_np, name="ident_const")
    ones_d = nc.inline_tensor(ones_np, name="ones_const")

    exp = mybir.ActivationFunctionType.Exp

    with tile.TileContext(nc) as tc:
        with (
            tc.tile_pool(name="sbin", bufs=1) as sbin,
            tc.tile_pool(name="sbu", bufs=6) as sbu,
            tc.tile_pool(name="sbe", bufs=3) as sbe,
            tc.tile_pool(name="ps_s", bufs=4, space="PSUM") as ps_s,
            tc.tile_pool(name="ps_o", bufs=1, space="PSUM") as ps_o,
            tc.tile_pool(name="ps_l", bufs=1, space="PSUM") as ps_l,
        ):
            # Critical-path DMAs first: K chunk 0 / first q half of the
            # first (longest) sequence, so the PE starts ~10us earlier.
            b0 = border[0]
            kt_t = [None] * B
            qt_t = [None] * B
            v_t = [None] * B
            w0 = cb[b0] * CHUNK
            kt_first = sbin.tile([D, w0], F32R, tag=f"kt{b0}")
            nc.sync.dma_start(
                kt_first[:, 0:CHUNK],
                kt_d.ap()[:, offs[b0] * CHUNK : offs[b0] * CHUNK + CHUNK],
            )
            qt_first = sbin.tile([D, QCOLS], F32R, tag=f"qt{b0}")
            nc.sync.dma_start(
                qt_first[:, 0:NHALF],
                qt_d.ap()[:, b0 * QCOLS : b0 * QCOLS + NHALF],
            )
            nc.sync.dma_start(
                qt_first[:, NHALF:QCOLS],
                qt_d.ap()[:, b0 * QCOLS + NHALF : (b0 + 1) * QCOLS],
            )
            v_first = sbin.tile([CHUNK, w0], F32R, tag=f"v{b0}")
            nc.sync.dma_start(
                v_first[:, 0 : 2 * CHUNK],
                v_d.ap()[:, offs[b0] * CHUNK : offs[b0] * CHUNK + 2 * CHUNK],
            )
            kcut = CHUNK
            vcut = 2 * CHUNK
            while kcut < w0 or vcut < w0:
                khi = min(kcut + 4 * CHUNK, w0)
                if khi > kcut:
                    nc.sync.dma_start(
                        kt_first[:, kcut:khi],
                        kt_d.ap()[
                            :, offs[b0] * CHUNK + kcut : offs[b0] * CHUNK + khi
                        ],
                    )
                    kcut = khi
                vhi = min(vcut + 4 * CHUNK, w0)
                if vhi > vcut:
                    nc.sync.dma_start(
                        v_first[:, vcut:vhi],
                        v_d.ap()[
                            :, offs[b0] * CHUNK + vcut : offs[b0] * CHUNK + vhi
                        ],
                    )
                    vcut = vhi
            kt_t[b0] = kt_first
            qt_t[b0] = qt_first

            identr = sbin.tile([CHUNK, CHUNK], mybir.dt.bfloat16, tag="identr")
            nc.sync.dma_start(identr[:], identb_d.ap())
            ones = sbin.tile([CHUNK, 2], F32R, tag="ones")
            nc.gpsimd.dma_start(ones[:], ones_d.ap())
            masks = sbin.tile([CHUNK, nmask * Q], mybir.dt.bfloat16, tag="masks")
            cut = Q * sum(
                1 for bb, _, _ in masked if cb[bb] <= cb[border[1]]
            )
            cut = max(Q, min(cut, nmask * Q))
            nc.sync.dma_start(masks[:, 0:cut], mask_d.ap()[:, 0:cut])
            if cut < nmask * Q:
                nc.sync.dma_start(
                    masks[:, cut : nmask * Q], mask_d.ap()[:, cut : nmask * Q]
                )

            for b in border:
                w = cb[b] * CHUNK
                head = min(2 * CHUNK, w)
                o0 = offs[b] * CHUNK
                if b == border[0]:
                    v_t[b] = v_first
                    continue
                vt = sbin.tile([CHUNK, w], F32R, tag=f"v{b}")
                if kt_t[b] is None:
                    kt = sbin.tile([D, w], F32R, tag=f"kt{b}")
                    nc.sync.dma_start(
                        kt[:, 0:head], kt_d.ap()[:, o0 : o0 + head]
                    )
                    qt = sbin.tile([D, QCOLS], F32R, tag=f"qt{b}")
                    nc.sync.dma_start(
                        qt[:], qt_d.ap()[:, b * QCOLS : (b + 1) * QCOLS]
                    )
                    nc.sync.dma_start(
                        vt[:, 0:head], v_d.ap()[:, o0 : o0 + head]
                    )
                    if head < w:
                        nc.sync.dma_start(
                            kt[:, head:w], kt_d.ap()[:, o0 + head : o0 + w]
                        )
                        nc.sync.dma_start(
                            vt[:, head:w], v_d.ap()[:, o0 + head : o0 + w]
                        )
                    kt_t[b] = kt
                    qt_t[b] = qt
                else:
                    nc.sync.dma_start(
                        vt[:, 0:head], v_d.ap()[:, o0 : o0 + head]
                    )
                    if head < w:
                        nc.sync.dma_start(
                            vt[:, head:w], v_d.ap()[:, o0 + head : o0 + w]
                        )
                v_t[b] = vt

            def half_state(b, c, n):
                # 'skip' = every q in the half is masked for this chunk;
                # 'mask' = the causal diagonal crosses this (chunk, half)
                lo = int(L[b]) - Q + n * CHUNK
                if c * CHUNK > lo + CHUNK - 1:
                    return "skip"
                if c * CHUNK + CHUNK - 1 > lo:
                    return "mask"
                return "clear"

            def emit_score(b, c):
                mi = mask_idx.get((b, c))
                u_h = []
                for n in range(2):
                    st = half_state(b, c, n)
                    if st == "skip":
                        u_h.append(None)
                        continue
                    s_ps = ps_s.tile([CHUNK, NHALF], F32, tag="s")
                    nc.tensor.matmul(
                        s_ps[:],
                        kt_t[b][:, c * CHUNK : (c + 1) * CHUNK],
                        qt_t[b][:, n * NHALF : (n + 1) * NHALF],
                        start=True,
                        stop=st == "clear",
                    )
                    if st == "mask":
                        mb = (
                            masks[
                                :,
                                mi * Q + n * CHUNK : mi * Q + (n + 1) * CHUNK,
                            ]
                            .unsqueeze(2)
                            .broadcast_to([CHUNK, CHUNK, G])
                        )
                        nc.tensor.matmul(
                            s_ps[:], identr[:], mb, start=False, stop=True
                        )
                    u = sbu.tile([CHUNK, NHALF], F32R, tag="u")
                    nc.scalar.activation(u[:], s_ps[:], exp, scale=SCALE)
                    u_h.append(u)
                return u_h

            def emit_consume(b, c, u_h, o_ps, l_ps, last_n):
                for n in range(2):
                    if u_h[n] is None:
                        continue
                    nc.tensor.matmul(
                        l_ps[:, n * NHALF : (n + 1) * NHALF],
                        ones[:, 0:2],
                        u_h[n][:],
                        start=c == 0,
                        stop=c == last_n[n],
                    )
                for n in range(2):
                    if u_h[n] is None:
                        continue
                    nc.tensor.matmul(
                        o_ps[:, n * NHALF : (n + 1) * NHALF],
                        v_t[b][:, c * CHUNK : (c + 1) * CHUNK],
                        u_h[n][:],
                        start=c == 0,
                        stop=c == last_n[n],
                    )

            u0_next = None
            for bi, b in enumerate(border):
                terminal = bi == len(border) - 1
                nchunks = cb[b]
                # last contributing chunk per half (later ones are skipped)
                last_n = [
                    min(nchunks - 1, (int(L[b]) - Q + n * CHUNK + CHUNK - 1) // CHUNK)
                    for n in range(2)
                ]
                o_ps = ps_o.tile([D, QCOLS], F32, tag="o")
                l_ps = ps_l.tile([2, QCOLS], F32, tag="l")
                for c in range(nchunks):
                    if c == 0 and u0_next is not None:
                        u_h = u0_next
                        u0_next = None
                    else:
                        u_h = emit_score(b, c)
                    emit_consume(b, c, u_h, o_ps, l_ps, last_n)

                # epilogue: rl = 1/l broadcast down partitions, one multiply.
                # o is copied out of PSUM immediately so the next sequence's
                # PV accumulation can claim the banks.
                l_sb = sbe.tile([1, QCOLS], F32, tag="lsb")
                if terminal:
                    # tail chain: l-copy on the (now idle) ScalarE, and read
                    # O straight from PSUM -- no next sequence needs the banks
                    nc.scalar.copy(l_sb[:], l_ps[0:1, :])
                    osrc = o_ps
                else:
                    nc.vector.tensor_copy(l_sb[:], l_ps[0:1, :])
                    ocp = sbe.tile([D, QCOLS], F32, tag="ocp")
                    nc.vector.tensor_copy(ocp[:], o_ps[:])
                    osrc = ocp
                rl_row = sbe.tile([1, QCOLS], F32, tag="rlrow")
                rlb = sbe.tile([D, QCOLS], F32, tag="rlb")
                out_sb = sbe.tile([D, QCOLS], F32, tag="osb")
                for n in range(2):
                    half = slice(n * NHALF, (n + 1) * NHALF)
                    nc.vector.reciprocal_approx_fast(
                        rl_row[:, half], l_sb[:, half]
                    )
                    nc.gpsimd.partition_broadcast(
                        rlb[:, half], rl_row[:, half]
                    )
                    nc.vector.tensor_mul(
                        out_sb[:, half], osrc[:, half], rlb[:, half]
                    )
                    nc.sync.dma_start(
                        out_d.ap()[b][:, half], out_sb[:, half]
                    )

    nc.compile()
    return nc, L, cb, offs


def _pack_inputs(query, k_cache, v_cache, block_tables, L, cb, offs):
    """Gather the paged cache and pack per-core shards in device layouts."""
    C = int(offs[-1])
    k_lin = k_cache[block_tables].reshape(B, KV, KVH, D)
    v_lin = v_cache[block_tables].reshape(B, KV, KVH, D)
    kt_all = np.zeros((KVH, D, C * CHUNK), dtype=np.float32)
    v_all = np.zeros((KVH, CHUNK, C * CHUNK), dtype=np.float32)
    for b in range(B):
        Lb, w = int(L[b]), cb[b] * CHUNK
        kk = np.zeros((w, KVH, D), dtype=np.float32)
        kk[:Lb] = k_lin[b, :Lb]
        # [w, KVH, D] -> [KVH, D, w]
        kt_all[:, :, offs[b] * CHUNK : offs[b] * CHUNK + w] = kk.transpose(
            1, 2, 0
        )
        vv = np.zeros((w, KVH, D), dtype=np.float32)
        vv[:Lb] = v_lin[b, :Lb]
        # [cb, 128, KVH, D] -> [KVH, 128, cb, D] -> [KVH, 128, w]
        v_all[:, :, offs[b] * CHUNK : offs[b] * CHUNK + w] = (
            vv.reshape(cb[b], CHUNK, KVH, D)
            .transpose(2, 1, 0, 3)
            .reshape(KVH, CHUNK, w)
        )
    # query [B,Q,H,D] -> [KVH, D, B, Q, G] (t-major, g inner)
    qt_all = (
        query.transpose(2, 3, 0, 1)
        .reshape(KVH, G, D, B, Q)
        .transpose(0, 2, 3, 4, 1)
        .reshape(KVH, D, B * QCOLS)
    )
    qt_all = np.ascontiguousarray(qt_all, dtype=np.float32)
    return [
        {
            "kt": np.ascontiguousarray(kt_all[h]),
            "v": np.ascontiguousarray(v_all[h]),
            "qt": qt_all[h],
        }
        for h in range(KVH)
    ]


def _unpack_outputs(results):
    """[B,D,QCOLS] per core (O^T, q=(g,t) on cols) -> [B*Q, H*D]."""
    out = np.empty((B * Q, H * D), dtype=np.float32)
    for h, res in enumerate(results):
        o = res["out"].reshape(B, D, Q, G)  # [b, d, t, g]
        o = o.transpose(0, 2, 3, 1).reshape(B * Q, G * D)
        out[:, h * G * D : (h + 1) * G * D] = o
    return out


def kernel(query, k_cache, v_cache, block_tables, seq_lens):
    query = np.asarray(query, dtype=np.float32)
    k_cache = np.asarray(k_cache, dtype=np.float32)
    v_cache = np.asarray(v_cache, dtype=np.float32)
    block_tables = np.asarray(block_tables, dtype=np.int64)
    nc, L, cb, offs = _build(np.asarray(seq_lens))
    in_maps = _pack_inputs(query, k_cache, v_cache, block_tables, L, cb, offs)
    res = run_bass_kernel_spmd(nc, in_maps, core_ids=list(range(N_CORES)))
    return _unpack_outputs(res.results)
